# revision 4
# baseline (speedup 1.0000x reference)
"""Conv1d (B=32, C_in=C_out=64, L=16384, K=3, VALID) on 8 trn2 cores.

Strategy: data-parallel over batch (4 batches/core), polyphase-2 over L.
The host de-interleaves each batch's length axis into even/odd phases
stacked across 128 SBUF partitions: rows 0-63 = x[c, 0::2], rows
64-127 = x[c, 1::2].  The K=3 conv then needs only TWO PSUM-accumulated
matmuls per output chunk (vs 3 for the tap-per-matmul scheme):

  out_even(m) = w0 Xe[m] + w1 Xo[m] + w2 Xe[m+1]
  out_odd(m)  = w0 Xo[m] + w1 Xe[m+1] + w2 Xo[m+1]

  pass A: rhs = [Xe;Xo][:, m],   lhsT_A = [[w0^T, 0   ], [w1^T, w0^T]]
  pass B: rhs = [Xe;Xo][:, m+1], lhsT_B = [[w2^T, w1^T], [0,    w2^T]]

PSUM [128, n] = [out_even ch; out_odd ch]; the host re-interleaves.
This cuts TensorE busy ~48us -> ~30us, taking it off the critical path.

The kernel is fabric-bound: input + output (16.9 MB fp16) share the
~435 GB/s SBUF-AXI DMA fabric, and the 16 SDMA engines round-robin
at packet granularity between the three DGE rings (sync-HWDGE,
scalar-HWDGE, gpsimd-SWDGE).  Ring assignment = bandwidth shares, so:
input chunks alternate sync/scalar (input must finish as early as
possible - it gates the whole tail), early outputs go on the SWDGE
ring, and late outputs cycle over all three rings so the post-input
drain is not capped by one ring's ~280 GB/s packet rate.  Input DMAs
are issued one chunk ahead of compute so the scalar-ring triggers are
not queued behind ACT's PSUM evacuations.  PSUM->SBUF evacuation
(fused bias add, fp32->fp16) alternates whole 512-col chunks between
ACT and DVE.  I/O is fp16 (~3e-4 rel err).  Shapes hardcoded.
"""

import os

import numpy as np

from concourse import bacc, bass, mybir, tile
from concourse.bass_utils import run_bass_kernel_spmd

B, C, L, K = 32, 64, 16384, 3
LOUT = L - K + 1  # 16382
NCORES = 8
BPC = B // NCORES  # 4 batches per core
P = 128  # partitions (2 phases x C)
LH = L // 2  # 8192 phase-cols per batch
MOUT = LOUT // 2  # 8191 output phase-cols per batch
NJ = 512  # PSUM inner chunk (one fp32 bank)

F32 = mybir.dt.float32
F16 = mybir.dt.float16

CH = int(os.environ.get("CONV_CH", "4096"))
IBUFS = int(os.environ.get("CONV_IBUFS", "10"))
OBUFS = int(os.environ.get("CONV_OBUFS", "8"))
WARMUP = int(os.environ.get("CONV_WARMUP", "8"))

_NC_CACHE = []


def _chunks():
    """Global chunk list [(batch, m0, n), ...]; each batch sums to MOUT.
    Batch 0 ramps up small so compute starts early; the last batch
    ramps down so the compute-gated tail after the final input is
    short."""
    ramp = [512, 1024, 2048]
    tail = [2048, 1024, 512, 511]
    out = []
    for p in range(BPC):
        if p == 0:
            rest = MOUT - sum(ramp)
            body = [CH] * (rest // CH)
            lst = ramp + body + [rest - sum(body)]
        elif p == BPC - 1:
            rest = MOUT - sum(tail)
            body = [CH] * (rest // CH)
            lst = body + [rest - sum(body)] + tail
        else:
            body = [CH] * (MOUT // CH)
            lst = body + [MOUT - sum(body)]
        lst = [n for n in lst if n > 0]
        assert sum(lst) == MOUT, (p, lst)
        m0 = 0
        for n in lst:
            out.append((p, m0, n))
            m0 += n
    return out


def _build_nc():
    nc = bacc.Bacc("TRN2", target_bir_lowering=False, debug=False,
                   num_devices=NCORES)

    x2 = nc.dram_tensor("x2", [BPC, P, LH], F16, kind="ExternalInput")
    wT = nc.dram_tensor("wT", [P, 2, P], F16, kind="ExternalInput")
    b2 = nc.dram_tensor("b2", [P, 1], F32, kind="ExternalInput")
    y2 = nc.dram_tensor("y2", [BPC, P, MOUT], F16, kind="ExternalOutput")

    chunks = _chunks()
    nch = len(chunks)
    in_engines = {}
    out_engines = {}
    for i, (p, m0, n) in enumerate(chunks):
        in_engines[i] = "sync" if i % 2 == 0 else "scalar"
        if p < 2:
            out_engines[i] = "gpsimd"
        else:
            out_engines[i] = ["gpsimd", "sync", "scalar"][i % 3]
    # the very last output should land on an otherwise-empty ring
    out_engines[nch - 1] = "scalar"
    out_engines[nch - 2] = "sync"
    out_engines[nch - 3] = "gpsimd"

    with tile.TileContext(nc) as tc:
        with (
            tc.tile_pool(name="const", bufs=1) as const_pool,
            tc.tile_pool(name="inp", bufs=IBUFS) as inp_pool,
            tc.tile_pool(name="outp", bufs=OBUFS) as outp_pool,
            tc.tile_pool(name="psum", bufs=8, space=bass.MemorySpace.PSUM)
            as psum_pool,
        ):
            def issue_in(i):
                p, m0, n = chunks[i]
                it = inp_pool.tile([P, CH + 1], F16, tag="in")
                eng = {"sync": nc.sync, "scalar": nc.scalar}[in_engines[i]]
                eng.dma_start(out=it[:, :n + 1], in_=x2[p, :, m0:m0 + n + 1])
                return it

            # First input chunk's DMA goes out before the const DMAs so
            # the input stream starts as early as possible.
            tiles = {0: issue_in(0)}

            w = const_pool.tile([P, 2, P], F16)
            nc.sync.dma_start(out=w[:], in_=wT[:])
            bias = const_pool.tile([P, 1], F32)
            nc.sync.dma_start(out=bias[:], in_=b2[:])
            tiles[1] = issue_in(1)

            # HAM warm-up: dummy matmuls on zeroed SBUF while the first
            # input DMA is in flight, so the PE clock gate is at 8/8
            # (2.4 GHz) when real work arrives.
            if WARMUP:
                wz = const_pool.tile([P, NJ], F16)
                nc.vector.memset(wz[:], 0.0)
                for i in range(WARMUP):
                    wp = psum_pool.tile([P, NJ], F32, tag="acc",
                                        name=f"warm{i}")
                    nc.tensor.matmul(wp[:], wz[:, :P], wz[:],
                                     start=True, stop=True)

            ci = 0  # global psum-chunk counter (ACT/DVE alternation)
            for i, (p, m0, n) in enumerate(chunks):
                it = tiles.pop(i)
                if i + 2 < nch:
                    tiles[i + 2] = issue_in(i + 2)
                ot = outp_pool.tile([P, CH], F16, tag="out")
                for j0 in range(0, n, NJ):
                    nj = min(NJ, n - j0)
                    pt = psum_pool.tile([P, NJ], F32, tag="acc")
                    nc.tensor.matmul(pt[:, :nj], w[:, 0, :],
                                     it[:, j0:j0 + nj],
                                     start=True, stop=False)
                    nc.tensor.matmul(pt[:, :nj], w[:, 1, :],
                                     it[:, j0 + 1:j0 + 1 + nj],
                                     start=False, stop=True)
                    # psum -> sbuf with fused bias add; whole chunk on
                    # one engine, alternating ACT/DVE
                    if ci % 2 == 0:
                        nc.scalar.add(ot[:, j0:j0 + nj], pt[:, :nj],
                                      add=bias[:, 0:1])
                    else:
                        nc.vector.tensor_scalar_add(ot[:, j0:j0 + nj],
                                                    pt[:, :nj],
                                                    bias[:, 0:1])
                    ci += 1
                eng = {"sync": nc.sync, "scalar": nc.scalar,
                       "gpsimd": nc.gpsimd}[out_engines[i]]
                eng.dma_start(out=y2[p, :, m0:m0 + n], in_=ot[:, :n])

    nc.compile()
    return nc


def _get_nc():
    if not _NC_CACHE:
        _NC_CACHE.append(_build_nc())
    return _NC_CACHE[0]


def _prep_weights(weight, bias):
    w = weight.astype(np.float32)
    wT = np.zeros((P, 2, P), np.float32)
    w0, w1, w2 = w[:, :, 0].T, w[:, :, 1].T, w[:, :, 2].T  # [C_in, C_out]
    wT[0:C, 0, 0:C] = w0
    wT[C:P, 0, 0:C] = w1
    wT[C:P, 0, C:P] = w0
    wT[0:C, 1, 0:C] = w2
    wT[0:C, 1, C:P] = w1
    wT[C:P, 1, C:P] = w2
    b2 = np.concatenate([bias, bias]).reshape(P, 1).astype(np.float32)
    return wT.astype(np.float16), b2


def kernel(x, weight, bias, _want_results=False, **run_kwargs):
    x = np.asarray(x, np.float32)
    weight = np.asarray(weight, np.float32)
    bias = np.asarray(bias, np.float32)
    nc = _get_nc()
    wT, b2 = _prep_weights(weight, bias)

    # de-interleave length into even/odd phases stacked on partitions
    xh = x.astype(np.float16)
    in_maps = []
    for i in range(NCORES):
        xs = xh[BPC * i:BPC * (i + 1)]  # [BPC, C, L]
        xde = np.empty((BPC, P, LH), np.float16)
        xde[:, :C, :] = xs[:, :, 0::2]
        xde[:, C:, :] = xs[:, :, 1::2]
        in_maps.append({"x2": xde, "wT": wT, "b2": b2})

    res = run_bass_kernel_spmd(nc, in_maps, list(range(NCORES)), **run_kwargs)

    out = np.empty((B, C, LOUT), np.float32)
    for i in range(NCORES):
        yde = res.results[i]["y2"]  # [BPC, P, MOUT] f16
        ob = out[BPC * i:BPC * (i + 1)]
        ob[:, :, 0::2] = yde[:, :C, :]
        ob[:, :, 1::2] = yde[:, C:, :]
    if _want_results:
        return out, res
    return out


# revision 5
# speedup vs baseline: 1.1497x; 1.1497x over previous
"""Conv1d (B=32, C_in=C_out=64, L=16384, K=3, VALID) on 8 trn2 cores.

Strategy: data-parallel over batch (4 batches/core), polyphase-2 over L.
The host de-interleaves each batch's length axis into even/odd phases
stacked across 128 SBUF partitions: rows 0-63 = x[c, 0::2], rows
64-127 = x[c, 1::2].  The K=3 conv then needs only TWO PSUM-accumulated
matmuls per output chunk (vs 3 for the tap-per-matmul scheme):

  out_even(m) = w0 Xe[m] + w1 Xo[m] + w2 Xe[m+1]
  out_odd(m)  = w0 Xo[m] + w1 Xe[m+1] + w2 Xo[m+1]

  pass A: rhs = [Xe;Xo][:, m],   lhsT_A = [[w0^T, 0   ], [w1^T, w0^T]]
  pass B: rhs = [Xe;Xo][:, m+1], lhsT_B = [[w2^T, w1^T], [0,    w2^T]]

PSUM [128, n] = [out_even ch; out_odd ch]; the host re-interleaves.
This cuts TensorE busy ~48us -> ~30us, taking it off the critical path.

The kernel is fabric-bound: input + output (16.9 MB fp16) share the
~435 GB/s SBUF-AXI DMA fabric; the 16 SDMA engines round-robin at
packet granularity between the three DGE rings (sync-HWDGE,
scalar-HWDGE, gpsimd-SWDGE), and a ring sustains only ~286 GB/s by
itself (per-engine inter-packet gap).  So:
 - One input tile per batch [128, 8192], filled by sub-DMAs whose
   per-partition rows are 4 KB multiples (or a single <=4KB packet).
   A 4097-col halo transfer would split rows into 4096+4096+2-byte
   packets; the 2-byte runt packets cost a full packet slot each and
   cap the ring at ~190 GB/s (measured).  Chunk-col reads that span
   sub-DMA boundaries are handled by Tile's subtile deps.
 - Input sub-DMAs alternate the sync/scalar rings (two rings: input
   must finish as early as possible since it gates the whole tail).
 - All input triggers are issued (program order) before any output
   trigger on the sync/scalar rings, so an output trigger waiting on
   evacuation can never head-of-line-block input prefetch.
 - Early outputs drain on the gpsimd SWDGE ring; late outputs cycle
   over sync/scalar/gpsimd so the post-input drain uses all rings.
PSUM->SBUF evacuation (fused bias add, fp32->fp16) alternates whole
512-col chunks between ACT and DVE.  I/O is fp16 (~3e-4 rel err).
"""

import os

import numpy as np

from concourse import bacc, bass, mybir, tile
from concourse.bass_utils import run_bass_kernel_spmd

B, C, L, K = 32, 64, 16384, 3
LOUT = L - K + 1  # 16382
NCORES = 8
BPC = B // NCORES  # 4 batches per core
P = 128  # partitions (2 phases x C)
LH = L // 2  # 8192 phase-cols per batch
MOUT = LOUT // 2  # 8191 output phase-cols per batch
NJ = 512  # PSUM inner chunk (one fp32 bank)

F32 = mybir.dt.float32
F16 = mybir.dt.float16

IBUFS = int(os.environ.get("CONV_IBUFS", "4"))
OBUFS = int(os.environ.get("CONV_OBUFS", "8"))
WARMUP = int(os.environ.get("CONV_WARMUP", "8"))

# input sub-DMA col counts per batch (rows are 4KB-multiples / <=4KB)
IN_SUBS = {0: [1024, 1024, 2048, 4096]}
for _p in range(1, BPC):
    IN_SUBS[_p] = [4096, 4096]
# output chunk col counts per batch (first batch staged smaller so the
# output stream starts early)
OUT_CHUNKS = {0: [2047, 2048, 4096]}
for _p in range(1, BPC):
    OUT_CHUNKS[_p] = [4095, 4096]
# output ring per global output-chunk index (late chunks fan out over
# all three rings; sync/scalar only after all input triggers)
OUT_ENG = ["gpsimd", "gpsimd", "gpsimd",
           "sync", "scalar", "gpsimd", "sync", "scalar", "gpsimd"]

_NC_CACHE = []


def _build_nc():
    nc = bacc.Bacc("TRN2", target_bir_lowering=False, debug=False,
                   num_devices=NCORES)

    x2 = nc.dram_tensor("x2", [BPC, P, LH], F16, kind="ExternalInput")
    wT = nc.dram_tensor("wT", [P, 2, P], F16, kind="ExternalInput")
    b2 = nc.dram_tensor("b2", [P, 1], F32, kind="ExternalInput")
    y2 = nc.dram_tensor("y2", [BPC, P, MOUT], F16, kind="ExternalOutput")

    with tile.TileContext(nc) as tc:
        with (
            tc.tile_pool(name="const", bufs=1) as const_pool,
            tc.tile_pool(name="inp", bufs=IBUFS) as inp_pool,
            tc.tile_pool(name="outp", bufs=OBUFS) as outp_pool,
            tc.tile_pool(name="psum", bufs=8, space=bass.MemorySpace.PSUM)
            as psum_pool,
        ):
            isel = [0]

            def issue_in(p, first=0):
                it = inp_pool.tile([P, LH], F16, tag="in")
                c0 = 0
                for si, n in enumerate(IN_SUBS[p]):
                    if si >= first:
                        eng = nc.sync if isel[0] % 2 == 0 else nc.scalar
                        eng.dma_start(out=it[:, c0:c0 + n],
                                      in_=x2[p, :, c0:c0 + n])
                        isel[0] += 1
                    c0 += n
                return it

            # batch 0's first two sub-DMAs lead everything (one per
            # HWDGE ring), then the consts, then the rest.
            it00 = inp_pool.tile([P, LH], F16, tag="in")
            nc.sync.dma_start(out=it00[:, 0:1024], in_=x2[0, :, 0:1024])
            nc.scalar.dma_start(out=it00[:, 1024:2048],
                                in_=x2[0, :, 1024:2048])
            w = const_pool.tile([P, 2, P], F16)
            nc.sync.dma_start(out=w[:], in_=wT[:])
            bias = const_pool.tile([P, 1], F32)
            nc.sync.dma_start(out=bias[:], in_=b2[:])
            nc.sync.dma_start(out=it00[:, 2048:4096], in_=x2[0, :, 2048:4096])
            nc.scalar.dma_start(out=it00[:, 4096:8192],
                                in_=x2[0, :, 4096:8192])
            tiles = {0: it00, 1: issue_in(1)}

            # HAM warm-up: dummy matmuls on zeroed SBUF while the first
            # input DMA is in flight, so the PE clock gate is at 8/8
            # (2.4 GHz) when real work arrives.
            if WARMUP:
                wz = const_pool.tile([P, NJ], F16)
                nc.vector.memset(wz[:], 0.0)
                for i in range(WARMUP):
                    wp = psum_pool.tile([P, NJ], F32, tag="acc",
                                        name=f"warm{i}")
                    nc.tensor.matmul(wp[:], wz[:, :P], wz[:],
                                     start=True, stop=True)

            ci = 0  # global psum-chunk counter (ACT/DVE alternation)
            oi = 0  # global output-chunk index (ring assignment)
            for p in range(BPC):
                if p + 2 < BPC:
                    tiles[p + 2] = issue_in(p + 2)
                it = tiles.pop(p)
                m0 = 0
                for n in OUT_CHUNKS[p]:
                    ot = outp_pool.tile([P, 4096], F16, tag="out")
                    for j0 in range(m0, m0 + n, NJ):
                        nj = min(NJ, m0 + n - j0)
                        o0 = j0 - m0
                        pt = psum_pool.tile([P, NJ], F32, tag="acc")
                        nc.tensor.matmul(pt[:, :nj], w[:, 0, :],
                                         it[:, j0:j0 + nj],
                                         start=True, stop=False)
                        nc.tensor.matmul(pt[:, :nj], w[:, 1, :],
                                         it[:, j0 + 1:j0 + 1 + nj],
                                         start=False, stop=True)
                        # psum -> sbuf with fused bias add; whole chunk
                        # on one engine, alternating ACT/DVE
                        if ci % 2 == 0:
                            nc.scalar.add(ot[:, o0:o0 + nj], pt[:, :nj],
                                          add=bias[:, 0:1])
                        else:
                            nc.vector.tensor_scalar_add(ot[:, o0:o0 + nj],
                                                        pt[:, :nj],
                                                        bias[:, 0:1])
                        ci += 1
                    eng = {"sync": nc.sync, "scalar": nc.scalar,
                           "gpsimd": nc.gpsimd}[OUT_ENG[oi]]
                    eng.dma_start(out=y2[p, :, m0:m0 + n], in_=ot[:, :n])
                    oi += 1
                    m0 += n

    nc.compile()
    return nc


def _get_nc():
    if not _NC_CACHE:
        _NC_CACHE.append(_build_nc())
    return _NC_CACHE[0]


def _prep_weights(weight, bias):
    w = weight.astype(np.float32)
    wT = np.zeros((P, 2, P), np.float32)
    w0, w1, w2 = w[:, :, 0].T, w[:, :, 1].T, w[:, :, 2].T  # [C_in, C_out]
    wT[0:C, 0, 0:C] = w0
    wT[C:P, 0, 0:C] = w1
    wT[C:P, 0, C:P] = w0
    wT[0:C, 1, 0:C] = w2
    wT[0:C, 1, C:P] = w1
    wT[C:P, 1, C:P] = w2
    b2 = np.concatenate([bias, bias]).reshape(P, 1).astype(np.float32)
    return wT.astype(np.float16), b2


def kernel(x, weight, bias, _want_results=False, **run_kwargs):
    x = np.asarray(x, np.float32)
    weight = np.asarray(weight, np.float32)
    bias = np.asarray(bias, np.float32)
    nc = _get_nc()
    wT, b2 = _prep_weights(weight, bias)

    # de-interleave length into even/odd phases stacked on partitions
    xh = x.astype(np.float16)
    in_maps = []
    for i in range(NCORES):
        xs = xh[BPC * i:BPC * (i + 1)]  # [BPC, C, L]
        xde = np.empty((BPC, P, LH), np.float16)
        xde[:, :C, :] = xs[:, :, 0::2]
        xde[:, C:, :] = xs[:, :, 1::2]
        in_maps.append({"x2": xde, "wT": wT, "b2": b2})

    res = run_bass_kernel_spmd(nc, in_maps, list(range(NCORES)), **run_kwargs)

    out = np.empty((B, C, LOUT), np.float32)
    for i in range(NCORES):
        yde = res.results[i]["y2"]  # [BPC, P, MOUT] f16
        ob = out[BPC * i:BPC * (i + 1)]
        ob[:, :, 0::2] = yde[:, :C, :]
        ob[:, :, 1::2] = yde[:, C:, :]
    if _want_results:
        return out, res
    return out
